# revision 34
# baseline (speedup 1.0000x reference)
"""Trainium2 Bass kernel for nn_BasicNet (CondConv 3-branch + BN + channel shuffle).

Reference computation:
  x [32, 256, 56, 56] split into 4 channel groups of 64:
    s0 passthrough,
    sq = BN(CondConv3x3(s1)), vr = BN(CondConv3x1(s2)), hz = BN(CondConv1x3(s3))
  out = channel_shuffle(concat([s0, sq, vr, hz]), groups=8)

Sharding: data-parallel over batch (4 samples per core on 8 cores); BN batch
stats (per-channel sum / sum-of-squares) are all-reduced across cores,
pipelined per branch.

v5 design (from HW profile of v4):
  - sample-pairing: each [128, *] tile holds sample A on partitions 0:64 and
    sample B on 64:128. Conv matmuls are K=64/M=64 diagonal PE tiles
    ((0,0) for A, (64,64) for B) that run concurrently in the array
    (measured ~110ns effective per MM pair element).
  - DMA latency is the v4 killer (~2-4us serialization per transfer on one
    ring): consts+weights packed into 2 DMAs, image loads issued first and
    split across the Sync and Scalar HWDGE rings, one merged store per
    (branch, pair) with channel shuffle + sample split folded into the dest
    AP, single DRAM->DRAM passthrough DMA.
  - all 6 attention chains run up front (pooled -> logits -> sigmoid ->
    masked-broadcast matmuls -> weight agg), so the conv MM stream is dense
    and ACT table switches are clustered.
  - branch order v, sq, h: the first (cheap) branch fires its AllReduce
    earliest, absorbing cross-core skew while sq convs still run.
"""

import sys

if '/opt/trn_rl_repo' not in sys.path:
    sys.path.insert(0, '/opt/trn_rl_repo')

import numpy as np
import ml_dtypes

import concourse.bass as bass
import concourse.bacc as bacc
import concourse.tile as tile
from concourse import mybir
from concourse import bass_utils

F32 = mybir.dt.float32
BF16 = mybir.dt.bfloat16

N_CORES = 8
NS = 4                   # samples per core
NPAIR = 2                # sample pairs per core
H = W = 56
HW = H * W               # 3136
C = 64                   # channels per branch (Cin == O == 64)
KEXP = 4                 # CondConv experts
ROWS_PER_TILE = 8
NT = ROWS_PER_TILE * W   # 448 free elements per matmul tile
N_TILES = H // ROWS_PER_TILE  # 7
M_TOTAL = 32 * HW        # BN stat count
EPS = 1e-5

# branch geometry: (name, (padded ph, pw), taps [(dy, dx)])
# order: v (cheap, AR first), sq, h
BR = [
    ('v', (58, 56), [(dy, 0) for dy in range(3)]),
    ('sq', (58, 58), [(dy, dx) for dy in range(3) for dx in range(3)]),
    ('h', (56, 58), [(0, dx) for dx in range(3)]),
]
X_SLICE = {0: 2, 1: 1, 2: 3}       # branch idx -> channel-group of x
G1 = {0: 4, 1: 2, 2: 6}            # branch idx -> first shuffle group g
NTAPS = [len(b[2]) for b in BR]
W_OFF = [0, NTAPS[0] * C, (NTAPS[0] + NTAPS[1]) * C]
W_COLS = sum(NTAPS) * C            # 960

# packed consts layout (f32, [128, CONST_W]):
#   att_w [12] | att_b [3] | diag_mask [4] | ones [64] | fold [64]
#   | dup [128] | gb [6]
CO_ATTW, CO_ATTB, CO_DIAG, CO_ONES, CO_FOLD, CO_DUP, CO_GB = \
    0, 12, 15, 19, 83, 147, 275
CONST_W = 281


def _build_nc():
    nc = bacc.Bacc('TRN2', target_bir_lowering=False, debug=False,
                   num_devices=N_CORES)

    x0 = nc.dram_tensor('x0', [NS, C, HW], F32, kind='ExternalInput').ap()
    xp = {}
    for bi, (bn, (ph, pw), taps) in enumerate(BR):
        xp[bi] = nc.dram_tensor(f'xp_{bn}', [NPAIR, 128, ph * pw], BF16,
                                kind='ExternalInput').ap()
    wall = nc.dram_tensor('wall', [128, KEXP, W_COLS], F32,
                          kind='ExternalInput').ap()
    cst = nc.dram_tensor('cst', [128, CONST_W], F32, kind='ExternalInput').ap()
    out = nc.dram_tensor('out', [NS, 4 * C, H, W], F32,
                         kind='ExternalOutput').ap()

    with tile.TileContext(nc) as tc:
        _emit(tc, x0, xp, wall, cst, out)

    nc.compile()
    return nc


def _emit(tc, x0, xp, wall, cst, out):
    nc = tc.nc
    from contextlib import ExitStack
    ctx = ExitStack()
    with ctx:
        persist = ctx.enter_context(tc.tile_pool(name='persist', bufs=1))
        smalls = ctx.enter_context(tc.tile_pool(name='smalls', bufs=4))
        aggp = ctx.enter_context(tc.tile_pool(name='aggp', bufs=2))
        bouncep = ctx.enter_context(tc.tile_pool(name='bouncep', bufs=3))
        psum_conv = ctx.enter_context(
            tc.tile_pool(name='psum_conv', bufs=5, space='PSUM'))
        psum_small = ctx.enter_context(
            tc.tile_pool(name='psum_small', bufs=3, space='PSUM'))
        dram = ctx.enter_context(tc.tile_pool(name='dram', bufs=1,
                                              space='DRAM'))

        # channel-shuffled output view: [n, g, c2, hw]; concat-ch = g*32+c2
        ov4 = out.rearrange('n (c2 g) h w -> n g c2 (h w)', g=8)

        # ---------- loads: images first, split across both HWDGE rings ----
        in_tiles = {}
        for bi, (bn, (ph, pw), taps) in enumerate(BR):
            for p in range(NPAIR):
                t = persist.tile([128, ph * pw], BF16, tag=f'in_{bi}_{p}',
                                 name=f'in_{bi}_{p}')
                in_tiles[(bi, p)] = t

        # first branch pair loads, then consts+weights, then the rest
        nc.sync.dma_start(out=in_tiles[(0, 0)], in_=xp[0][0])
        nc.scalar.dma_start(out=in_tiles[(0, 1)], in_=xp[0][1])
        cst_sb = persist.tile([128, CONST_W], F32, tag='cst_sb')
        nc.sync.dma_start(out=cst_sb, in_=cst)
        w_sb = persist.tile([128, KEXP, W_COLS], F32, tag='w_sb')
        nc.scalar.dma_start(out=w_sb, in_=wall)
        for bi in (1, 2):
            nc.sync.dma_start(out=in_tiles[(bi, 0)], in_=xp[bi][0])
            nc.scalar.dma_start(out=in_tiles[(bi, 1)], in_=xp[bi][1])

        co = {
            'att_w': cst_sb[:, CO_ATTW:CO_ATTB].rearrange(
                'q (b k) -> q b k', b=3),
            'att_b': cst_sb[:, CO_ATTB:CO_DIAG],
            'diag_mask': cst_sb[:, CO_DIAG:CO_ONES],
            'ones': cst_sb[:, CO_ONES:CO_FOLD],
            'fold_mask': cst_sb[:, CO_FOLD:CO_DUP],
            'dup_mask': cst_sb[0:C, CO_DUP:CO_GB],
            'gb': cst_sb[0:C, CO_GB:CO_GB + 6].rearrange(
                'c (a b) -> c a b', a=2),
        }
        epst = persist.tile([C, 1], F32, tag='epst')
        nc.vector.memset(epst, EPS)

        # s0 passthrough: DRAM->DRAM on the scalar ring, one DMA per sample
        for s_ in range(NS):
            nc.scalar.dma_start(out=ov4[s_, 0:2], in_=x0[s_])

        # conv outputs (bf16) per (branch, pair)
        otiles = {}
        for bi in range(3):
            for p in range(NPAIR):
                otiles[(bi, p)] = persist.tile(
                    [128, HW], BF16, tag=f'ot_{bi}_{p}', name=f'ot_{bi}_{p}')

        bnst = {(bi, p): persist.tile([128, N_TILES, 6], F32,
                                      tag=f'bnst_{bi}_{p}',
                                      name=f'bnst_{bi}_{p}')
                for bi in range(3) for p in range(NPAIR)}
        red = {bi: persist.tile([128, NPAIR, 2], F32, tag=f'red_{bi}',
                                name=f'red_{bi}')
               for bi in range(3)}
        aggr = {}
        cc_in = {bi: dram.tile([C, 2], F32, name=f'cc_in_{bi}')
                 for bi in range(3)}
        cc_out = {bi: dram.tile([C, 2], F32, name=f'cc_out_{bi}')
                  for bi in range(3)}

        def att_agg(bi, p):
            """Attention + per-pair weight aggregation for pair (bi, p)."""
            bn, (ph, pw), taps = BR[bi]
            it = in_tiles[(bi, p)]
            ntap = len(taps)

            pooled = smalls.tile([128, 1], F32, tag='pooled', name='pooled')
            if p == 0:
                pscr = aggp.tile([128, 3364], BF16, tag='pscr', name='pscr')
                nc.scalar.activation(out=pscr[:, 0:ph * pw], in_=it,
                                     func=mybir.ActivationFunctionType.Copy,
                                     accum_out=pooled)
            else:
                nc.vector.tensor_reduce(out=pooled, in_=it,
                                        axis=mybir.AxisListType.X,
                                        op=mybir.AluOpType.add)
            att_ps = psum_small.tile([128, KEXP], F32, tag='sm', name='att_ps')
            nc.tensor.matmul(att_ps[0:KEXP, 0:1], lhsT=co['att_w'][0:C, bi, :],
                             rhs=pooled[0:C, :], start=True, stop=True)
            nc.tensor.matmul(att_ps[C:C + KEXP, 0:1],
                             lhsT=co['att_w'][C:128, bi, :],
                             rhs=pooled[C:128, :], start=True, stop=True)
            att_s = smalls.tile([128, 1], F32, tag='att_s', name='att_s')
            nc.scalar.activation(out=att_s, in_=att_ps[:, 0:1],
                                 func=mybir.ActivationFunctionType.Sigmoid,
                                 bias=co['att_b'][:, bi:bi + 1])
            diag = smalls.tile([128, KEXP], F32, tag='diag', name='diag')
            nc.vector.tensor_scalar_mul(out=diag, in0=co['diag_mask'],
                                        scalar1=att_s)
            bc_ps = psum_small.tile([128, KEXP], F32, tag='sm', name='bc_ps')
            nc.tensor.matmul(bc_ps[0:C, :], lhsT=co['ones'][0:KEXP, :],
                             rhs=diag[0:KEXP, :], start=True, stop=True)
            nc.tensor.matmul(bc_ps[C:128, :], lhsT=co['ones'][C:C + KEXP, :],
                             rhs=diag[C:C + KEXP, :], start=True, stop=True)
            att_bc = smalls.tile([128, KEXP], F32, tag='att_bc', name='att_bc')
            nc.scalar.activation(out=att_bc, in_=bc_ps,
                                 func=mybir.ActivationFunctionType.Copy)

            w4 = w_sb[:, :, W_OFF[bi]:W_OFF[bi] + ntap * C]
            agg = aggp.tile([128, ntap * C], F32, tag=f'agg_{bi}', name='agg')
            nc.vector.tensor_scalar_mul(out=agg, in0=w4[:, 0],
                                        scalar1=att_bc[:, 0:1])
            for k in range(1, KEXP - 1):
                nc.vector.scalar_tensor_tensor(
                    out=agg, in0=w4[:, k], scalar=att_bc[:, k:k + 1],
                    in1=agg, op0=mybir.AluOpType.mult, op1=mybir.AluOpType.add)
            agg_r = persist.tile([128, ntap * C], BF16, tag=f'aggr_{bi}_{p}',
                                 name=f'aggr_{bi}_{p}')
            nc.vector.scalar_tensor_tensor(
                out=agg_r, in0=w4[:, KEXP - 1],
                scalar=att_bc[:, KEXP - 1:KEXP], in1=agg,
                op0=mybir.AluOpType.mult, op1=mybir.AluOpType.add)
            aggr[(bi, p)] = agg_r

        def conv_pair(bi, p):
            """CondConv matmuls + evac + stats for sample pair p of branch."""
            bn, (ph, pw), taps = BR[bi]
            it = in_tiles[(bi, p)]
            ot = otiles[(bi, p)]
            ntap = len(taps)
            agg_r = aggr[(bi, p)]

            it3 = it.rearrange('c (r q) -> c r q', q=pw)
            for t in range(N_TILES):
                pt = psum_conv.tile([128, NT], F32, tag='pt', name='pt')
                r0 = ROWS_PER_TILE * t
                for j, (dy, dx) in enumerate(taps):
                    st, sp = (j == 0), (j == ntap - 1)
                    nc.tensor.matmul(
                        pt[0:C, :], lhsT=agg_r[0:C, j * C:(j + 1) * C],
                        rhs=it3[0:C, r0 + dy:r0 + dy + ROWS_PER_TILE,
                                dx:dx + W],
                        start=st, stop=sp, skip_group_check=True)
                    nc.tensor.matmul(
                        pt[C:128, :], lhsT=agg_r[C:128, j * C:(j + 1) * C],
                        rhs=it3[C:128, r0 + dy:r0 + dy + ROWS_PER_TILE,
                                dx:dx + W],
                        start=st, stop=sp, skip_group_check=True)
                dst = ot[:, t * NT:(t + 1) * NT]
                nc.scalar.activation(out=dst, in_=pt,
                                     func=mybir.ActivationFunctionType.Copy)
                nc.vector.bn_stats(out=bnst[(bi, p)][:, t, :], in_=dst)

            # pair stats -> (sum, sumsq) per partition
            mv = smalls.tile([128, 2], F32, tag='mv', name='mv')
            nc.vector.bn_aggr(out=mv, in_=bnst[(bi, p)])
            r = red[bi]
            nc.vector.tensor_scalar_mul(out=r[:, p, 0:1], in0=mv[:, 0:1],
                                        scalar1=float(HW))
            tmp = smalls.tile([128, 1], F32, tag='tmp_q', name='tmp_q')
            nc.vector.tensor_tensor(out=tmp, in0=mv[:, 0:1], in1=mv[:, 0:1],
                                    op=mybir.AluOpType.mult)
            nc.vector.tensor_tensor(out=tmp, in0=tmp, in1=mv[:, 1:2],
                                    op=mybir.AluOpType.add)
            nc.vector.tensor_scalar_mul(out=r[:, p, 1:2], in0=tmp,
                                        scalar1=float(HW))

        def branch_reduce(bi):
            """Fold partition halves + pairs, stage, all-reduce branch bi."""
            fold_ps = psum_small.tile([C, 2 * NPAIR], F32, tag='sm',
                                      name='fold_ps')
            nc.tensor.matmul(fold_ps, lhsT=co['fold_mask'],
                             rhs=red[bi].rearrange('q p s -> q (p s)'),
                             start=True, stop=True)
            fold_sb = smalls.tile([C, 2 * NPAIR], F32, tag='fold_sb',
                                  name='fold_sb')
            nc.scalar.activation(out=fold_sb, in_=fold_ps,
                                 func=mybir.ActivationFunctionType.Copy)
            cc_sb = smalls.tile([C, 2], F32, tag='cc_sb', name='cc_sb')
            nc.vector.tensor_tensor(out=cc_sb, in0=fold_sb[:, 0:2],
                                    in1=fold_sb[:, 2:4],
                                    op=mybir.AluOpType.add)
            nc.gpsimd.dma_start(out=cc_in[bi], in_=cc_sb)
            nc.gpsimd.collective_compute(
                'AllReduce', mybir.AluOpType.add,
                replica_groups=[list(range(N_CORES))],
                ins=[cc_in[bi].opt()], outs=[cc_out[bi].opt()])

        def branch_norm_store(bi):
            """Post-AR: scale/bias, normalize + store both pairs of branch."""
            ar_sb = smalls.tile([C, 2], F32, tag='ar_sb', name='ar_sb')
            nc.gpsimd.dma_start(out=ar_sb, in_=cc_out[bi])
            mv = smalls.tile([C, 2], F32, tag='mv2', name='mv2')
            nc.vector.tensor_scalar_mul(out=mv, in0=ar_sb,
                                        scalar1=1.0 / M_TOTAL)
            var = smalls.tile([C, 1], F32, tag='var', name='var')
            nc.vector.tensor_tensor(out=var, in0=mv[:, 0:1], in1=mv[:, 0:1],
                                    op=mybir.AluOpType.mult)
            nc.vector.tensor_tensor(out=var, in0=mv[:, 1:2], in1=var,
                                    op=mybir.AluOpType.subtract)
            sd = smalls.tile([C, 1], F32, tag='sd', name='sd')
            nc.scalar.activation(out=sd, in_=var,
                                 func=mybir.ActivationFunctionType.Sqrt,
                                 bias=epst)
            nc.vector.reciprocal(out=sd, in_=sd)
            sb2 = smalls.tile([C, 2], F32, tag='sb2', name='sb2')
            nc.vector.tensor_tensor(out=sb2[:, 0:1], in0=co['gb'][:, 0, bi:bi + 1],
                                    in1=sd, op=mybir.AluOpType.mult)
            tmpb = smalls.tile([C, 1], F32, tag='tmpb', name='tmpb')
            nc.vector.tensor_tensor(out=tmpb, in0=mv[:, 0:1], in1=sb2[:, 0:1],
                                    op=mybir.AluOpType.mult)
            nc.vector.tensor_tensor(out=sb2[:, 1:2], in0=co['gb'][:, 1, bi:bi + 1],
                                    in1=tmpb, op=mybir.AluOpType.subtract)
            dup_ps = psum_small.tile([128, 2], F32, tag='sm', name='dup_ps')
            nc.tensor.matmul(dup_ps, lhsT=co['dup_mask'], rhs=sb2,
                             start=True, stop=True)
            sb128 = persist.tile([128, 2], F32, tag=f'sb128_{bi}',
                                 name=f'sb128_{bi}')
            nc.scalar.activation(out=sb128, in_=dup_ps,
                                 func=mybir.ActivationFunctionType.Copy)

            g1 = G1[bi]
            for p in range(NPAIR):
                bounce = bouncep.tile([128, HW], F32, tag='bounce',
                                      name=f'bounce_{bi}_{p}')
                ot = otiles[(bi, p)]
                if p == 0:
                    nc.scalar.activation(
                        out=bounce, in_=ot,
                        func=mybir.ActivationFunctionType.Identity,
                        bias=sb128[:, 1:2], scale=sb128[:, 0:1])
                else:
                    nc.vector.tensor_scalar(
                        out=bounce, in0=ot, scalar1=sb128[:, 0:1],
                        scalar2=sb128[:, 1:2], op0=mybir.AluOpType.mult,
                        op1=mybir.AluOpType.add)
                # one store per sample: dest [2 groups, 32, HW]
                for s_ in range(2):
                    nc.scalar.dma_start(
                        out=ov4[2 * p + s_, g1:g1 + 2],
                        in_=bounce[C * s_:C * s_ + C, :])

        # ---------- schedule ----------
        for bi in range(3):
            att_agg(bi, 0)
            att_agg(bi, 1)
        conv_pair(0, 0)
        conv_pair(0, 1)
        branch_reduce(0)
        conv_pair(1, 0)
        conv_pair(1, 1)
        branch_reduce(1)
        branch_norm_store(0)
        conv_pair(2, 0)
        conv_pair(2, 1)
        branch_reduce(2)
        branch_norm_store(1)
        branch_norm_store(2)


_NC_CACHE = None


def _get_nc():
    global _NC_CACHE
    if _NC_CACHE is None:
        _NC_CACHE = _build_nc()
    return _NC_CACHE


def _host_weights(inputs):
    """All branches -> [128, K, W_COLS] f32 lhsT layout, dup halves."""
    wt = np.zeros((128, KEXP, W_COLS), np.float32)
    for bi, (bn, _, taps) in enumerate(BR):
        w = np.asarray(inputs[f'w_{bn}'], dtype=np.float32)
        k, o, cin, kh, kw = w.shape
        for j, (dy, dx) in enumerate(taps):
            blk = w[:, :, :, dy if kh > 1 else 0, dx if kw > 1 else 0]
            blk = blk.transpose(2, 0, 1)  # [cin, k, o]
            c0 = W_OFF[bi] + j * C
            wt[0:C, :, c0:c0 + C] = blk
            wt[C:128, :, c0:c0 + C] = blk
    return np.ascontiguousarray(wt)


def _prep_in_maps(inputs):
    x = np.ascontiguousarray(inputs['x'], dtype=np.float32)
    n_total = x.shape[0]
    pads = {0: (1, 0), 1: (1, 1), 2: (0, 1)}
    xpad = []
    for bi, (bn, (ph, pw), taps) in enumerate(BR):
        ph_, pw_ = pads[bi]
        sl = x[:, C * X_SLICE[bi]:C * (X_SLICE[bi] + 1)]  # [N, 64, H, W]
        p = np.zeros((n_total // 2, 128, ph, pw), ml_dtypes.bfloat16)
        sl2 = sl.reshape(n_total // 2, 2, C, H, W)
        p[:, 0:C, ph_:ph_ + H, pw_:pw_ + W] = sl2[:, 0]
        p[:, C:128, ph_:ph_ + H, pw_:pw_ + W] = sl2[:, 1]
        xpad.append(np.ascontiguousarray(p.reshape(n_total // 2, 128, ph * pw)))
    x0_full = np.ascontiguousarray(x[:, 0:C].reshape(n_total, C, HW))

    cstv = np.zeros((128, CONST_W), np.float32)
    for bi, (bn, _, _) in enumerate(BR):
        aw = np.asarray(inputs[f'att_w_{bn}'], np.float32).T / float(HW)
        cstv[0:C, CO_ATTW + 4 * bi:CO_ATTW + 4 * bi + 4] = aw
        cstv[C:128, CO_ATTW + 4 * bi:CO_ATTW + 4 * bi + 4] = aw
        ab = np.asarray(inputs[f'att_b_{bn}'], np.float32)
        cstv[:, CO_ATTB + bi] = ab[np.arange(128) % KEXP]
        cstv[0:C, CO_GB + 0 * 3 + bi] = np.asarray(inputs[f'g_{bn}'],
                                                   np.float32)
        cstv[0:C, CO_GB + 1 * 3 + bi] = np.asarray(inputs[f'b_{bn}'],
                                                   np.float32)
    for j in range(KEXP):
        cstv[j, CO_DIAG + j] = 1.0
        cstv[C + j, CO_DIAG + j] = 1.0
    cstv[:, CO_ONES:CO_FOLD] = 1.0
    cstv[np.arange(128), CO_FOLD + np.arange(128) % C] = 1.0
    cstv[np.arange(C), CO_DUP + np.arange(C)] = 1.0
    cstv[np.arange(C), CO_DUP + C + np.arange(C)] = 1.0

    shared = {'wall': _host_weights(inputs), 'cst': cstv}

    in_maps = []
    for ci in range(N_CORES):
        m = dict(shared)
        m['x0'] = x0_full[ci * NS:(ci + 1) * NS]
        for bi, (bn, _, _) in enumerate(BR):
            m[f'xp_{bn}'] = xpad[bi][ci * NPAIR:(ci + 1) * NPAIR]
        in_maps.append(m)
    return in_maps


def run_raw(inputs, trace=False, **kwargs):
    """Build+run; returns (full_output, BassKernelResults)."""
    nc = _get_nc()
    in_maps = _prep_in_maps(inputs)
    res = bass_utils.run_bass_kernel_spmd(
        nc, in_maps, core_ids=list(range(N_CORES)), trace=trace, **kwargs)
    full = np.concatenate([res.results[i]['out'] for i in range(N_CORES)], axis=0)
    return full, res


def kernel(**inputs):
    full, _ = run_raw(inputs)
    return full
